# revision 25
# baseline (speedup 1.0000x reference)
"""GQA kernel for trn2, 8 cores: DP over batch (2) x TP over kv-head groups (4).

Each core computes, for its (batch b, kv-group g):
  - qkv projection for its 4 q-heads + 1 kv-head (q pre-scaled by 1/sqrt(dk))
  - RoPE on q/k
  - full (non-causal) attention for the 4 q-heads vs its kv-head
  - partial out-projection with its 2048 rows of W_out
Host sums the 4 per-group partials per batch and adds bias.

Matmul operands are bf16 (PE runs fp32 at 1/4 rate; bf16 is full rate).
Accumulation and softmax statistics stay fp32.

Performance structure (measured ~662us/core vs 833us baseline):
  - x is transposed + all tensors pre-arranged on the host into
    partition-major SBUF layouts, so the PE does no transposes and every
    DMA moves 4KB+ contiguous runs per partition.
  - Softmax denominator: DVE adds over key chunks + one [1,512]
    ones-matmul per pair (instead of a 16-deep ones-matmul chain on PE).
  - Phase B interleaves V chains between QK chains so every PSUM drain
    (DVE rope) has >=2 chains of slack: no PE bubbles, and busy streaks
    stay >3us so the PE clock (DVFS) never downshifts.
  - Startup DMAs are scheduled just-in-time across the three DGE queues
    (sync/scalar HWDGE, gpsimd SWDGE) in exactly the order the PE
    consumes them (k-projection weights first).

Self-contained: hardcodes all shapes. kernel(**inputs) -> np.ndarray.
"""

import math
from contextlib import ExitStack

import numpy as np
import ml_dtypes

import concourse.bass as bass
import concourse.bacc as bacc
import concourse.tile as tile
import concourse.mybir as mybir
from concourse.bass_utils import run_bass_kernel_spmd

F32 = mybir.dt.float32
BF16 = mybir.dt.bfloat16
L = 2048          # sequence length
D = 2048          # d_model
DK = 128          # head dim (q/k)
DV = 512          # head dim (v)
NHQ = 4           # q heads per core
NI = 4            # query chunks of 512
NJ = 16           # key chunks of 128
NDCH = 16         # d_model chunks of 128

_NC_CACHE = {}


def build_nc():
    if "nc" in _NC_CACHE:
        return _NC_CACHE["nc"]
    nc = bacc.Bacc("TRN2", target_bir_lowering=False, debug=False)

    # all inputs are pre-arranged on the host into partition-major SBUF
    # layouts so every DMA is 4KB+ contiguous per partition
    xt_d = nc.dram_tensor("xtp", [NI, 128, NDCH, 512], BF16, kind="ExternalInput")
    wqk_d = nc.dram_tensor("wqkp", [5, 128, NDCH, 128], BF16, kind="ExternalInput")
    wv_d = nc.dram_tensor("wvp", [128, NDCH, DV], BF16, kind="ExternalInput")
    wo_d = nc.dram_tensor("wop", [4, 128, NDCH, 512], BF16, kind="ExternalInput")
    cos_d = nc.dram_tensor("cost", [DK, L], F32, kind="ExternalInput")
    sin_d = nc.dram_tensor("sint", [DK, L], F32, kind="ExternalInput")
    out_d = nc.dram_tensor("out", [L, D], F32, kind="ExternalOutput")

    EXP = mybir.ActivationFunctionType.Exp

    with ExitStack() as ctx:
        tc = ctx.enter_context(tile.TileContext(nc))
        # pools
        persist = ctx.enter_context(tc.tile_pool(name="persist", bufs=1))
        psS = ctx.enter_context(tc.tile_pool(name="psS", bufs=5, space="PSUM"))
        psA = ctx.enter_context(tc.tile_pool(name="psA", bufs=3, space="PSUM"))

        ones = persist.tile([128, 1], BF16)
        nc.vector.memset(ones, 1.0)

        qT = persist.tile([128, NHQ, L], BF16)      # [dk, h, pos]
        kT = persist.tile([128, L], BF16)           # [dk, pos]
        v_sb = persist.tile([128, NJ, DV], BF16)    # [key_in_chunk, key_chunk, e]

        # ---------------- Phase B: qkv projection + rope --------------------
        with tc.tile_pool(name="pb1", bufs=1) as pb1, \
             tc.tile_pool(name="pb2", bufs=2) as pb2:
            wv_sb = pb1.tile([128, NDCH, DV], BF16)
            # qk projection weights: loaded ONCE (shared across all i),
            # one per-head-column tile at a time, k (c=4) first, so each
            # chain's weights land exactly when the PE reaches that chain
            wqk_sb = pb1.tile([128, 5, NDCH, 128], BF16)

            def load_xcs(i, x_engs=None, rope_eng=None):
                """Prefetch x^T + rope-table chunks for query chunk i."""
                isl = slice(i * 512, (i + 1) * 512)
                # steady state: x rides sync (fastest, idle after startup) +
                # scalar; gpsimd (slowest SWDGE) carries no steady x traffic
                x_engs = x_engs or [nc.sync, nc.scalar, nc.sync, nc.scalar]
                xT = pb2.tile([128, NDCH, 512], BF16, tag="xT")
                for tg in range(4):
                    x_engs[tg].dma_start(
                        out=xT[:, 4 * tg:4 * tg + 4, :],
                        in_=xt_d.ap()[i][:, 4 * tg:4 * tg + 4, :])
                rope_eng = rope_eng or nc.scalar
                csc = pb2.tile([128, 512], F32, tag="cos")
                rope_eng.dma_start(out=csc, in_=cos_d.ap()[:, isl])
                ssc = pb2.tile([128, 512], F32, tag="sin")
                rope_eng.dma_start(out=ssc, in_=sin_d.ap()[:, isl])
                return xT, csc, ssc

            # startup: just-in-time across the three DMA queues, ordered by
            # when the PE consumes each piece.  Measured queue rates are
            # sync~128GB/s, scalar~74, gpsimd~47, so the critical prefix
            # (k-weights + x0) leans on sync/scalar.
            nc.sync.dma_start(out=wqk_sb[:, 4, 0:8, :],
                              in_=wqk_d.ap()[4][:, 0:8, :])
            nc.gpsimd.dma_start(out=wqk_sb[:, 4, 8:16, :],
                                in_=wqk_d.ap()[4][:, 8:16, :])
            cur = load_xcs(0,
                           x_engs=[nc.sync, nc.scalar, nc.sync, nc.scalar])
            for c in range(4):
                nc.sync.dma_start(out=wqk_sb[:, c], in_=wqk_d.ap()[c])
            wv_engs = [nc.scalar, nc.scalar, nc.gpsimd, nc.gpsimd]
            for tg in range(4):
                wv_engs[tg].dma_start(
                    out=wv_sb[:, 4 * tg:4 * tg + 4, :],
                    in_=wv_d.ap()[:, 4 * tg:4 * tg + 4, :])

            for i in range(NI):
                isl = slice(i * 512, (i + 1) * 512)
                xT, csc, ssc = cur
                if i < NI - 1:
                    cur = load_xcs(i + 1)

                def qk_chain(c):
                    # q/k projection + rope (c = 0..3 q heads, c = 4 is k)
                    ps = psA.tile([128, 512], F32, tag="acc")
                    for t in range(NDCH):
                        nc.tensor.matmul(
                            ps, lhsT=wqk_sb[:, c, t, :],
                            rhs=xT[:, t, :],
                            start=(t == 0), stop=(t == NDCH - 1))
                    dest = qT[:, c, isl] if c < NHQ else kT[:, isl]
                    tmp = pb2.tile([128, 512], F32, tag="rope")
                    nc.vector.tensor_mul(tmp[0:64, :], ps[64:128, :], ssc[0:64, :])
                    nc.vector.tensor_mul(tmp[64:128, :], ps[0:64, :], ssc[64:128, :])
                    tmp2 = pb2.tile([128, 512], F32, tag="rope2")
                    nc.vector.tensor_mul(tmp2, ps, csc)
                    nc.vector.tensor_sub(dest[0:64, :], tmp2[0:64, :], tmp[0:64, :])
                    nc.vector.tensor_add(dest[64:128, :], tmp2[64:128, :],
                                         tmp[64:128, :])

                def v_chain(lsub):
                    ps = psA.tile([128, 512], F32, tag="acc")
                    for t in range(NDCH):
                        nc.tensor.matmul(
                            ps, lhsT=xT[:, t, lsub * 128:(lsub + 1) * 128],
                            rhs=wv_sb[:, t, :],
                            start=(t == 0), stop=(t == NDCH - 1))
                    nc.scalar.copy(out=v_sb[:, i * 4 + lsub, :], in_=ps)

                # k first (phase C waits on it); V chains interleaved between
                # QK chains so every PSUM drain (DVE rope, ~3us) has >=2
                # chains (~7us) of slack -> no PE bubbles, DVFS stays high.
                # For i=0 the V chains sit later so wv's DMA has time to land.
                if i == 0:
                    order = [(qk_chain, 4), (qk_chain, 0), (qk_chain, 1),
                             (qk_chain, 2), (qk_chain, 3), (v_chain, 0),
                             (v_chain, 1), (v_chain, 2), (v_chain, 3)]
                else:
                    order = [(qk_chain, 4), (qk_chain, 0), (qk_chain, 1),
                             (v_chain, 0), (qk_chain, 2), (v_chain, 1),
                             (qk_chain, 3), (v_chain, 2), (v_chain, 3)]
                for fn, arg in order:
                    fn(arg)

        # ---------------- Phase C+D: attention + fused out-projection -------
        # software-pipelined: S/exp of pair k+1 is emitted before ones/PV of
        # pair k so ACT exp latency hides under PE's PV matmuls.
        with tc.tile_pool(name="pc1", bufs=1) as pc1, \
             tc.tile_pool(name="pc2", bufs=2) as pc2:
            ctxTs = {}

            def emit_s_exp(i, h):
                isl = slice(i * 512, (i + 1) * 512)
                expS = pc2.tile([128, NJ, 512], BF16, tag="expS")
                sacc = pc2.tile([128, 512], BF16, tag="sacc")
                for j in range(NJ):
                    ps = psS.tile([128, 512], F32, tag="stream")
                    nc.tensor.matmul(ps, lhsT=kT[:, j * 128:(j + 1) * 128],
                                     rhs=qT[:, h, isl])
                    nc.scalar.activation(out=expS[:, j, :], in_=ps, func=EXP)
                    # running denominator partial sum on DVE (free axis = j)
                    if j == 1:
                        nc.vector.tensor_add(sacc, expS[:, 0, :], expS[:, 1, :])
                    elif j > 1:
                        nc.vector.tensor_add(sacc, sacc, expS[:, j, :])
                return expS, sacc

            def emit_pv(i, h, expS, sacc):
                # cross-partition part of the denominator: one 512-row matmul
                pso = psA.tile([1, 512], F32, tag="acc")
                nc.tensor.matmul(pso, lhsT=ones[:, 0:1], rhs=sacc)
                recip = pc1.tile([1, 512], F32, tag="recip")
                nc.vector.reciprocal(recip, pso)
                rb = pc2.tile([128, 512], F32, tag="rb")
                nc.gpsimd.partition_broadcast(rb, recip)
                ctxT = pc1.tile([128, 4, 512], BF16, tag=f"ctx{h}")
                for ec in range(4):
                    ps = psA.tile([128, 512], F32, tag="acc")
                    for j in range(NJ):
                        nc.tensor.matmul(
                            ps, lhsT=v_sb[:, j, ec * 128:(ec + 1) * 128],
                            rhs=expS[:, j, :],
                            start=(j == 0), stop=(j == NJ - 1))
                    nc.vector.tensor_mul(ctxT[:, ec, :], ps, rb)
                ctxTs[h] = ctxT

            def emit_outproj(i):
                for dm in range(4):
                    wo_t = pc2.tile([128, NDCH, 512], BF16, tag="wo")
                    for half in range(2):
                        nc.sync.dma_start(
                            out=wo_t[:, 8 * half:8 * half + 8, :],
                            in_=wo_d.ap()[dm][:, 8 * half:8 * half + 8, :])
                    for lsub in range(4):
                        l0 = i * 512 + lsub * 128
                        # the very last tile is computed as two half-width
                        # chains so the final store's drain latency (copy +
                        # DMA of the trailing half) is halved
                        halves = ([slice(0, 256), slice(256, 512)]
                                  if (i, dm, lsub) == (3, 3, 3)
                                  else [slice(0, 512)])
                        for hsl in halves:
                            ps = psA.tile([128, hsl.stop - hsl.start], F32,
                                          tag="acc")
                            for h in range(NHQ):
                                for ec in range(4):
                                    t = h * 4 + ec
                                    nc.tensor.matmul(
                                        ps,
                                        lhsT=ctxTs[h][:, ec,
                                                      lsub * 128:
                                                      (lsub + 1) * 128],
                                        rhs=wo_t[:, t, hsl],
                                        start=(t == 0), stop=(t == 15))
                            ost = pc2.tile([128, hsl.stop - hsl.start], F32,
                                           tag="ost")
                            nc.scalar.copy(out=ost, in_=ps)
                            nc.scalar.dma_start(
                                out=out_d.ap()[l0:l0 + 128,
                                               dm * 512 + hsl.start:
                                               dm * 512 + hsl.stop],
                                in_=ost)

            pairs = [(i, h) for i in range(NI) for h in range(NHQ)]
            prev = None
            for (i, h) in pairs:
                cur = (i, h, *emit_s_exp(i, h))
                if prev is not None:
                    pi, ph, pexp, psacc = prev
                    emit_pv(pi, ph, pexp, psacc)
                    if ph == NHQ - 1:
                        emit_outproj(pi)
                prev = cur
            pi, ph, pexp, psacc = prev
            emit_pv(pi, ph, pexp, psacc)
            emit_outproj(pi)

    nc.compile()
    _NC_CACHE["nc"] = nc
    return nc


def make_core_inputs(x, W_attn, W_out):
    """Split full inputs into 8 per-core input maps (core = b*4 + g)."""
    Q_DIM = 2048
    K_DIM = 512
    scale = np.float32(1.0 / math.sqrt(DK))
    bf = ml_dtypes.bfloat16

    # rope tables, mirroring the fp32 reference computation
    inv_freq = (np.float32(1.0) /
                (np.float32(10000.0) **
                 (np.arange(0, DK, 2, dtype=np.float32) / np.float32(DK))))
    freqs = np.arange(L, dtype=np.float32)[:, None] * inv_freq[None, :]  # [L,64]
    ang = np.concatenate([freqs, freqs], axis=-1)  # [L, 128]
    cosT = np.ascontiguousarray(np.cos(ang).T.astype(np.float32))  # [128, L]
    sinT = np.ascontiguousarray(np.sin(ang).T.astype(np.float32))

    # x pre-arranged as [i, p, t, l]: xtp[i,p,t,l] = x[b][i*512+l, t*128+p]
    xts = [np.ascontiguousarray(
        x[b].reshape(NI, 512, NDCH, 128).transpose(0, 3, 2, 1)).astype(bf)
        for b in range(2)]

    in_maps = []
    for core in range(8):
        b, g = divmod(core, 4)
        wq = (W_attn[:, 512 * g:512 * (g + 1)] * scale)
        wk = W_attn[:, Q_DIM + 128 * g:Q_DIM + 128 * (g + 1)]
        wqk = np.concatenate([wq, wk], axis=1)  # [2048, 640]
        # [c, p, t, cc]: wqkp[c,p,t,cc] = wqk[t*128+p, c*128+cc]
        wqkp = np.ascontiguousarray(
            wqk.reshape(NDCH, 128, 5, 128).transpose(2, 1, 0, 3)).astype(bf)
        wv = W_attn[:, Q_DIM + K_DIM + 512 * g:Q_DIM + K_DIM + 512 * (g + 1)]
        # [p, t, e]
        wvp = np.ascontiguousarray(
            wv.reshape(NDCH, 128, DV).transpose(1, 0, 2)).astype(bf)
        wo = W_out[2048 * g:2048 * (g + 1), :]  # [2048, 2048]
        # [dm, p, t, cc]: wop[dm,p,t,cc] = wo[t*128+p, dm*512+cc]
        wop = np.ascontiguousarray(
            wo.reshape(NDCH, 128, 4, 512).transpose(2, 1, 0, 3)).astype(bf)
        in_maps.append({
            "xtp": xts[b],
            "wqkp": wqkp,
            "wvp": wvp,
            "wop": wop,
            "cost": cosT,
            "sint": sinT,
        })
    return in_maps


def kernel(x, W_attn, W_out, b_out, _trace=False, _trace_cores=None):
    x = np.asarray(x)
    W_attn = np.asarray(W_attn)
    W_out = np.asarray(W_out)
    b_out = np.asarray(b_out)
    nc = build_nc()
    in_maps = make_core_inputs(x, W_attn, W_out)
    res = run_bass_kernel_spmd(
        nc, in_maps, core_ids=list(range(8)),
        trace=_trace, trace_cores=_trace_cores)
    parts = [res.results[c]["out"] for c in range(8)]
    out = np.empty((2, L, D), dtype=np.float32)
    for b in range(2):
        acc = parts[4 * b].astype(np.float32)
        for g in range(1, 4):
            acc = acc + parts[4 * b + g]
        out[b] = acc + b_out[None, :].astype(np.float32)
    if _trace:
        kernel._last_results = res
    return out
